# revision 2
# baseline (speedup 1.0000x reference)
"""Trainium2 Bass kernel for nn_ColorResBlock (LayerNorm + color-structured conv + ReLU residual).

Math (reference):
    h  = LayerNorm(x) over last axis (3072), affine (gamma,beta) tied per k across 6 colors
    xr = h.reshape(B, 24, 512, 6)
    out[b,i,o,d] = sum_k alpha[o,k] xr[b,i,k,d] + sum_k beta[o,k] xsum[b,i,k] + bias[o]
    result = x + relu(out)

Key algebraic restructuring (all folds computed on host):
    A2[o,k]  = (W0-W1)[o,k] * gamma[k]
    C1[o]    = -(sum_k (W0-W1)[o,k] g[k]) - 6*sum_k W1[o,k] g[k]
    B3[o,k]  = W1[o,k]*g[k] + C1[o]/3072          (folds the mean*C1 rank-1 term)
    C0[o]    = (W0-W1)@bn + 6*(W1@bn) + bias
    h_d[r,o] = rstd[r] * ( A2 @ x_d[r] + B3 @ xsum[r] + std[r]*C0[o] )
so the matmuls run on RAW x (de-interleaved+transposed on device via PE), and the
per-row layernorm enters only as a per-partition scale in the epilogue.

Data parallel over batch: 8 cores x 128 batches.  Per core: rows R=3072, channels 3072.
"""

import numpy as np
import concourse.bacc as bacc
import concourse.bass as bass
import concourse.tile as tile
from concourse import mybir, bass_utils

F32 = mybir.dt.float32
F32R = mybir.dt.float32r
BF16 = mybir.dt.bfloat16
AF = mybir.ActivationFunctionType
OP = mybir.AluOpType

B, NPOS, KH, NCOL = 1024, 24, 512, 6
CORES = 8
BL = B // CORES            # batches per core
R = BL * NPOS              # 3072 rows per core
C = KH * NCOL              # 3072 channels
P = 128                    # partitions per row tile
RT = R // P                # 24 row tiles
KC = KH // P               # 4 contraction chunks of 128
EPS = 1e-5

# d-plane -> engine for the PSUM->SBUF transpose copies ("dve" or "act")
COPY_ENG = ["dve", "act", "dve", "act", "dve", "act"]
RELU_ENG = "act"           # "act" | "dve"
RESID_ENG = "gpsimd"       # "gpsimd" | "dve"

_CACHE = {}


def _build(use_c0: bool):
    nc = bacc.Bacc("TRN2", target_bir_lowering=False, debug=False)
    x_d = nc.dram_tensor("x", [R, C], F32, kind="ExternalInput").ap()
    a2_d = nc.dram_tensor("a2t", [P, KC, KH], F32, kind="ExternalInput").ap()
    b3_d = nc.dram_tensor("b3t", [P, KC, KH], F32, kind="ExternalInput").ap()
    id_d = nc.dram_tensor("ident", [P, P], F32, kind="ExternalInput").ap()
    c0_d = nc.dram_tensor("c0row", [1, KH], F32, kind="ExternalInput").ap()
    out_d = nc.dram_tensor("out", [R, C], F32, kind="ExternalOutput").ap()

    with tile.TileContext(nc) as tc:
        with tc.tile_pool(name="wgt", bufs=1) as wgt, \
             tc.tile_pool(name="big", bufs=2) as big, \
             tc.tile_pool(name="med", bufs=2) as med, \
             tc.tile_pool(name="pln", bufs=3) as pln, \
             tc.tile_pool(name="sml", bufs=2) as sml, \
             tc.tile_pool(name="pT", bufs=2, space="PSUM") as pT, \
             tc.tile_pool(name="pXS", bufs=2, space="PSUM") as pXS, \
             tc.tile_pool(name="pY", bufs=2, space="PSUM") as pY, \
             tc.tile_pool(name="pZ", bufs=1, space="PSUM") as pZ:

            # ---- one-time setup: weights + identity rounded to f32r
            stage = wgt.tile([P, KC, KH], F32, tag="stage")
            nc.sync.dma_start(out=stage, in_=a2_d)
            a2t = wgt.tile([P, KC, KH], F32R)
            nc.vector.tensor_copy(out=a2t, in_=stage)
            stage2 = wgt.tile([P, KC, KH], F32, tag="stage")
            nc.sync.dma_start(out=stage2, in_=b3_d)
            b3t = wgt.tile([P, KC, KH], F32R)
            nc.vector.tensor_copy(out=b3t, in_=stage2)
            idstage = wgt.tile([P, P], F32, tag="idstage")
            nc.sync.dma_start(out=idstage, in_=id_d)
            identr = wgt.tile([P, P], F32R)
            nc.vector.tensor_copy(out=identr, in_=idstage)
            eps_t = wgt.tile([P, 1], F32)
            nc.vector.memset(eps_t, EPS)
            if use_c0:
                c0row = wgt.tile([1, KH], F32)
                nc.sync.dma_start(out=c0row, in_=c0_d)
                identf = wgt.tile([P, P], F32)
                nc.vector.tensor_copy(out=identf, in_=idstage)
            trash = wgt.tile([P, C], BF16, tag="trash")

            x3_d = x_d.rearrange("(t p) c -> t p c", p=P)
            o3_d = out_d.rearrange("(t p) c -> t p c", p=P)

            for t in range(RT):
                # ---- load
                x_t = big.tile([P, C], F32, tag="x")
                nc.sync.dma_start(out=x_t, in_=x3_d[t])

                # ---- cast to f32r (rounding pass) + row sums on the side
                xr_t = big.tile([P, C], F32R, tag="xr")
                sum_x = sml.tile([P, 1], F32, tag="sum")
                nc.vector.tensor_scalar(out=xr_t, in0=x_t, scalar1=0.0, scalar2=0.0,
                                        op0=OP.add, op1=OP.add, accum_out=sum_x)
                # ---- sum of squares (ACT square pass; output discarded)
                sumsq = sml.tile([P, 1], F32, tag="sumsq")
                nc.scalar.activation(out=trash, in_=x_t, func=AF.Square,
                                     accum_out=sumsq)
                # ---- stats -> rstd (and std if C0 used)
                mean_t = sml.tile([P, 1], F32, tag="mean")
                nc.vector.tensor_scalar(out=mean_t, in0=sum_x, scalar1=1.0 / C,
                                        scalar2=None, op0=OP.mult)
                e2_t = sml.tile([P, 1], F32, tag="e2")
                nc.vector.tensor_scalar(out=e2_t, in0=sumsq, scalar1=1.0 / C,
                                        scalar2=None, op0=OP.mult)
                nvar_t = sml.tile([P, 1], F32, tag="nvar")
                nc.vector.scalar_tensor_tensor(out=nvar_t, in0=mean_t, scalar=mean_t,
                                               in1=e2_t, op0=OP.mult, op1=OP.subtract)
                std_t = sml.tile([P, 1], F32, tag="std")
                nc.scalar.activation(out=std_t, in_=nvar_t, func=AF.Sqrt,
                                     bias=eps_t, scale=-1.0)
                rstd_t = sml.tile([P, 1], F32, tag="rstd")
                nc.vector.reciprocal(out=rstd_t, in_=std_t)

                xr3 = xr_t.rearrange("p (k d) -> p k d", d=NCOL)

                # ---- per-color transpose (PE, f32r) + Y matmuls, software-pipelined
                planes = []
                psums_y = []

                def emit_trans(d):
                    ps = pT.tile([P, KH], F32R, tag="pT")
                    for kc in range(KC):
                        nc.tensor.matmul(ps[:, kc * P:(kc + 1) * P],
                                         xr3[:, kc * P:(kc + 1) * P, d], identr,
                                         is_transpose=True)
                    pl = pln.tile([P, KH], F32R, tag="plane")
                    if COPY_ENG[d] == "dve":
                        nc.vector.tensor_copy(out=pl, in_=ps)
                    else:
                        nc.scalar.copy(out=pl, in_=ps)
                    planes.append(pl)

                def emit_y(d):
                    acc = pY.tile([P, KH], F32, tag="pY")
                    for kc in range(KC):
                        nc.tensor.matmul(acc, planes[d][:, kc * P:(kc + 1) * P],
                                         a2t[:, kc],
                                         start=(kc == 0),
                                         stop=(kc == KC - 1 and not use_c0))
                    psums_y.append(acc)

                emit_trans(0)
                emit_trans(1)
                for d in range(NCOL):
                    if d + 2 < NCOL:
                        emit_trans(d + 2)
                    emit_y(d)

                # ---- xsum^T accumulated over colors (PE transpose accumulate, f32 adds)
                psxs = pXS.tile([P, KH], F32R, tag="pXS")
                for kc in range(KC):
                    for d in range(NCOL):
                        nc.tensor.matmul(psxs[:, kc * P:(kc + 1) * P],
                                         xr3[:, kc * P:(kc + 1) * P, d], identr,
                                         is_transpose=True,
                                         start=(d == 0), stop=(d == NCOL - 1))
                xspl = med.tile([P, KH], F32R, tag="xsplane")
                nc.scalar.copy(out=xspl, in_=psxs)

                # ---- Z = B3 @ xsum^T  (shared across colors), scaled by rstd
                accz = pZ.tile([P, KH], F32, tag="pZ")
                for kc in range(KC):
                    nc.tensor.matmul(accz, xspl[:, kc * P:(kc + 1) * P], b3t[:, kc],
                                     start=(kc == 0), stop=(kc == KC - 1))
                s_scaled = med.tile([P, KH], F32, tag="sscaled")
                nc.vector.tensor_scalar(out=s_scaled, in0=accz, scalar1=rstd_t,
                                        scalar2=None, op0=OP.mult)

                # ---- optional rank-1 std*C0 term into each Y psum
                if use_c0:
                    pstd = pXS.tile([1, P], F32, tag="pstdT")
                    nc.tensor.matmul(pstd, std_t, identf, is_transpose=True)
                    stdT = sml.tile([1, P], F32, tag="stdT")
                    nc.vector.tensor_copy(out=stdT, in_=pstd)
                    for d in range(NCOL):
                        nc.tensor.matmul(psums_y[d], stdT, c0row,
                                         start=False, stop=True)

                # ---- epilogue: h_d = rstd*Y_d + s_scaled -> interleave, relu, +x
                out_t = big.tile([P, C], F32, tag="out")
                o3 = out_t.rearrange("p (k d) -> p k d", d=NCOL)
                for d in range(NCOL):
                    nc.vector.scalar_tensor_tensor(out=o3[:, :, d], in0=psums_y[d],
                                                   scalar=rstd_t, in1=s_scaled,
                                                   op0=OP.mult, op1=OP.add)
                if RELU_ENG == "act":
                    nc.scalar.activation(out=out_t, in_=out_t, func=AF.Relu)
                else:
                    nc.vector.tensor_scalar(out=out_t, in0=out_t, scalar1=0.0,
                                            scalar2=None, op0=OP.max)
                if RESID_ENG == "gpsimd":
                    nc.gpsimd.tensor_tensor(out=out_t, in0=out_t, in1=x_t, op=OP.add)
                else:
                    nc.vector.tensor_tensor(out=out_t, in0=out_t, in1=x_t, op=OP.add)
                nc.scalar.dma_start(out=o3_d[t], in_=out_t)

    nc.compile()
    return nc


def _host_prep(conv_weight, conv_bias, norm_weight, norm_bias):
    w = conv_weight.astype(np.float64)
    g = norm_weight.astype(np.float64)
    bn = norm_bias.astype(np.float64)
    alpha = w[:, :, 0] - w[:, :, 1]
    betaw = w[:, :, 1]
    a2 = alpha * g[None, :]
    c1 = -(alpha @ g) - NCOL * (betaw @ g)
    b3 = betaw * g[None, :] + c1[:, None] / C
    c0 = alpha @ bn + NCOL * (betaw @ bn) + conv_bias.astype(np.float64)

    def chunk_kT(m):  # [o,k] -> [128, KC, o] with tile[p,kc,o] = m[o, kc*128+p]
        return np.ascontiguousarray(
            m.T.reshape(KC, P, KH).transpose(1, 0, 2).astype(np.float32))

    a2t = chunk_kT(a2)
    b3t = chunk_kT(b3)
    c0row = np.ascontiguousarray(c0.astype(np.float32).reshape(1, KH))
    use_c0 = bool(np.any(c0row != 0.0))
    ident = np.eye(P, dtype=np.float32)
    return a2t, b3t, c0row, use_c0, ident


def _run(inputs, trace=False):
    x = inputs["x"]
    a2t, b3t, c0row, use_c0, ident = _host_prep(
        inputs["conv_weight"], inputs["conv_bias"],
        inputs["norm_weight"], inputs["norm_bias"])
    key = use_c0
    if key not in _CACHE:
        _CACHE[key] = _build(use_c0)
    nc = _CACHE[key]

    shards = x.reshape(CORES, BL * NPOS, C).astype(np.float32, copy=False)
    in_maps = [{"x": np.ascontiguousarray(shards[i]), "a2t": a2t, "b3t": b3t,
                "ident": ident, "c0row": c0row} for i in range(CORES)]
    res = bass_utils.run_bass_kernel_spmd(nc, in_maps, list(range(CORES)),
                                          trace=trace)
    out = np.concatenate([r["out"].reshape(1, BL, NPOS, C) for r in res.results],
                         axis=0).reshape(B, NPOS, C)
    return out, res


def kernel(x, conv_weight, conv_bias, norm_weight, norm_bias):
    out, _ = _run({"x": x, "conv_weight": conv_weight, "conv_bias": conv_bias,
                   "norm_weight": norm_weight, "norm_bias": norm_bias})
    return out
